# revision 36
# baseline (speedup 1.0000x reference)
"""Trainium2 Bass kernel for nn_Attention_45835890982922.

Dense multi-head attention block:
    qkv = x @ w_qkv ; q,k layernormed per head (eps=1e-5), q scaled by D^-0.5
    out = softmax(q k^T) v ; y = concat_heads(out) @ w_proj + b_proj

Sharding over 8 NeuronCores: hybrid batch x tensor-parallel.
Core c handles batch b = c//2 and heads [6*(c%2), 6*(c%2)+6).
Each core computes a partial y^T (its 6 heads through the matching
w_proj rows); the host sums the two partials per batch and adds b_proj.

On-chip layout is feature-major (transposed): the host ships x^T per
core, so every matmul contraction lives on the partition axis with no
on-chip transposes.  Softmax runs without max-subtraction (|S| <= ~8
after LN), with the normalization sum obtained from an extra all-ones
column appended to v; the division is folded into the epilogue of the
attention-output matmul.

Pipeline structure (per iteration):
  A: x^T streamed in 512-token quarters; each quarter does qkv matmuls,
     LN stat matmuls, and v matmuls before the next quarter's DMA lands.
  C: rstd chain + LN apply in place.
  D+E fused: one flat software-pipelined stream over (query-chunk,
     head, key-tile) slots; PV matmuls trail S/exp by PVLAG slots,
     continuously across unit boundaries, so ACT always has exp work
     while the PE alternates S and PV.  LN-apply groups and projection
     half-tiles ride the per-slot PE slack as filler thunks; the last
     chunk's projection spills into the next iteration's phase A.

All SBUF pools are persistent across repeat iterations so steady-state
runs prefetch the next iteration's x while attention drains.

dtypes: float32r (TensorE reduced fp32, ~1.5e-4) for qkv/S/stats/proj
matmuls, bf16 for exp(S) probabilities and v, fp32 accumulation in PSUM.
"""

from contextlib import ExitStack

import numpy as np

import concourse.bacc as bacc
import concourse.tile as tile
import concourse.mybir as mybir
from concourse.bass_utils import run_bass_kernel_spmd

F32 = mybir.dt.float32
F32R = mybir.dt.float32r
BF16 = mybir.dt.bfloat16
OP = mybir.AluOpType
AF = mybir.ActivationFunctionType

B, N, C, H, D = 4, 2048, 768, 12, 64
HL = H // 2              # 6 heads per core
CL = HL * D              # 384 local feature rows
P = 128
NKT = N // P             # 16 key tiles
QC = 1024                # query chunk for attention
NQC = N // QC
CT = C // P              # 6 contraction tiles over C
FT_QK = 2 * CL // P      # 6 feature tiles for q|k
KT3 = CL // P            # 3 contraction tiles over CL
LN_EPS = 1e-5
SCALE = float(D) ** -0.5
XQ = 512                 # x^T streaming quarter (tokens)
NXQ = N // XQ
PVLAG = 2                # PV runs this many key-tiles behind S/exp


def _build(trivial_beta: bool, repeat: int = 1):
    nc = bacc.Bacc("TRN2", target_bir_lowering=False, debug=False, num_devices=8)

    xT_d = nc.dram_tensor("xT", [C, N], F32R, kind="ExternalInput").ap()
    wqk_d = nc.dram_tensor("wqk", [C, 2 * CL], F32R, kind="ExternalInput").ap()
    wv_d = nc.dram_tensor("wv", [C, CL], F32R, kind="ExternalInput").ap()
    wp_d = nc.dram_tensor("wp", [CL, C], F32R, kind="ExternalInput").ap()
    bd6_d = nc.dram_tensor("bd6", [CL, 32], F32R, kind="ExternalInput").ap()
    bc12_d = nc.dram_tensor("bc12", [64, FT_QK * P], F32R, kind="ExternalInput").ap()
    gb_d = nc.dram_tensor("gb", [CL, 4], F32, kind="ExternalInput").ap()
    y_d = nc.dram_tensor("y", [C, N], F32, kind="ExternalOutput").ap()

    with tile.TileContext(nc) as tc, ExitStack() as top:
        top.enter_context(
            nc.allow_low_precision(reason="f32r/bf16 staging is intentional")
        )
        const = top.enter_context(tc.tile_pool(name="const", bufs=1))
        bd6 = const.tile([P, KT3, 32], F32R)
        nc.sync.dma_start(bd6[:], bd6_d.rearrange("(t p) h -> p t h", p=P))
        bc12 = const.tile([64, FT_QK * P], F32R)
        nc.sync.dma_start(bc12[:], bc12_d)
        gb = const.tile([P, KT3, 4], F32)
        nc.sync.dma_start(gb[:], gb_d.rearrange("(t p) c -> p t c", p=P))

        # persistent state: weights (loaded once), qk/v/out accumulators,
        # LN stats, x^T streaming ring, exp ring, epilogue smalls.
        pers = top.enter_context(tc.tile_pool(name="pers", bufs=1))
        wqk_r = pers.tile([P, CT, 2 * CL], F32R)
        nc.sync.dma_start(wqk_r[:], wqk_d.rearrange("(t p) f -> p t f", p=P))
        wv_r = pers.tile([P, CT, CL], F32R)
        nc.sync.dma_start(wv_r[:], wv_d.rearrange("(t p) f -> p t f", p=P))
        wp_r = pers.tile([P, KT3, C], F32R)
        nc.sync.dma_start(wp_r[:], wp_d.rearrange("(t p) f -> p t f", p=P))

        # v token-major bf16 with per-head all-ones column: [p, ttile, h*65+e]
        v_sb = pers.tile([P, NKT, HL * 65], BF16)
        v_view = v_sb[:].rearrange("p t (h e) -> p t h e", h=HL)
        nc.gpsimd.memset(v_view[:, :, :, 64:65], 1.0)

        qk_fts = [pers.tile([P, N], F32R, name=f"qk_ft{ft}") for ft in range(FT_QK)]
        out_fts = [pers.tile([P, N], F32R, name=f"out_ft{t}") for t in range(KT3)]
        # combined stats: rows 0-5 = q heads, rows 32-37 = k heads; the
        # unused middle rows are zeroed once so full-partition reads (chain
        # ops, bcast matmuls against zero bc12 rows) never see garbage.
        sm_mu = pers.tile([64, N], F32R, name="sm_mu")
        sm_rst = pers.tile([64, N], F32R, name="sm_rst")


        xtp = top.enter_context(tc.tile_pool(name="xtp", bufs=2))
        sqp = top.enter_context(tc.tile_pool(name="sqp", bufs=2))
        expp = top.enter_context(tc.tile_pool(name="expp", bufs=4 + PVLAG))
        epi = top.enter_context(tc.tile_pool(name="epi", bufs=1))
        cwork = top.enter_context(tc.tile_pool(name="cwork", bufs=2))

        state = dict(
            xT_d=xT_d, y_d=y_d, bd6=bd6, bc12=bc12, gb=gb,
            wqk_r=wqk_r, wv_r=wv_r, wp_r=wp_r,
            v_view=v_view, qk_fts=qk_fts, out_fts=out_fts,
            sm_mu=sm_mu, sm_rst=sm_rst,
            xtp=xtp, sqp=sqp, expp=expp, epi=epi, cwork=cwork,
        )
        pending = None
        for rep in range(repeat):
            pending = _emit_iteration(nc, tc, rep, trivial_beta, state, pending)
        with tc.tile_pool(name="psT", bufs=1, space="PSUM") as psT:
            for t in pending:
                t(psT)

    nc.compile()
    return nc


class _FtView:
    """view[p_slice, ft, col_slice] -> per-ft tile AP."""
    def __init__(self, tiles):
        self.tiles = tiles

    def __getitem__(self, idx):
        p, ft, col = idx
        return self.tiles[ft][p, col]


def _emit_iteration(nc, tc, rep, trivial_beta, st, pending=None):
    xT_d, y_d = st["xT_d"], st["y_d"]
    bd6, bc12, gb = st["bd6"], st["bc12"], st["gb"]
    wqk_r, wv_r, wp_r = st["wqk_r"], st["wv_r"], st["wp_r"]
    v_view = st["v_view"]
    sm_mu, sm_rst = st["sm_mu"], st["sm_rst"]
    xtp, sqp, expp, epi = st["xtp"], st["sqp"], st["expp"], st["epi"]
    cwork = st["cwork"]
    qk_raw = _FtView(st["qk_fts"])
    hat = qk_raw
    out_t = _FtView(st["out_fts"])

    # ================ phase A: x^T quarters -> qk, LN stats, v ================
    with ExitStack() as sA:
        psA = sA.enter_context(tc.tile_pool(name=f"psA{rep}", bufs=2, space="PSUM"))
        for g in range(NXQ):
            gs = slice(g * XQ, (g + 1) * XQ)
            x_t = xtp.tile([P, CT, XQ], F32R, tag="xt", name=f"xt_{rep}_{g}")
            nc.sync.dma_start(
                x_t[:], xT_d.rearrange("(t p) n -> p t n", p=P)[:, :, gs]
            )
            for ft in range(FT_QK):
                ps = psA.tile([P, XQ], F32, tag="ps_qkv")
                for kt in range(CT):
                    nc.tensor.matmul(
                        ps[:],
                        wqk_r[:, kt, ft * P:(ft + 1) * P],
                        x_t[:, kt, :],
                        start=(kt == 0),
                        stop=(kt == CT - 1),
                    )
                nc.vector.tensor_copy(qk_raw[:, ft, gs], ps[:])
                if pending:
                    pending.pop(0)(psA)
            # LN stat sums over D via block-diagonal ones matmuls; q and k
            # accumulate in separate base-0 PSUM tiles, merged at the copy.
            for s in range(2):
                psm = psA.tile([32, XQ], F32, tag="ps_stat")
                for kt in range(KT3):
                    nc.tensor.matmul(
                        psm[:],
                        bd6[:, kt, :],
                        qk_raw[:, 3 * s + kt, gs],
                        start=(kt == 0),
                        stop=(kt == KT3 - 1),
                    )
                nc.vector.tensor_scalar_mul(
                    sm_mu[32 * s:32 * s + 32, gs], psm[:], 1.0 / D
                )
            for s in range(2):
                psm2 = psA.tile([32, XQ], F32, tag="ps_stat")
                for kt in range(KT3):
                    sq = sqp.tile([P, XQ], F32R, tag="sq", name=f"sq_{rep}_{g}_{s}_{kt}")
                    nc.scalar.square(sq[:], qk_raw[:, 3 * s + kt, gs])
                    nc.tensor.matmul(
                        psm2[:],
                        bd6[:, kt, :],
                        sq[:],
                        start=(kt == 0),
                        stop=(kt == KT3 - 1),
                    )
                nc.vector.tensor_scalar_mul(
                    sm_rst[32 * s:32 * s + 32, gs], psm2[:], 1.0 / D
                )
            # v matmuls for this quarter's token tiles
            for tq in range(XQ // P):
                tt = g * (XQ // P) + tq
                psv = psA.tile([P, CL], F32, tag="ps_v")
                for kt in range(CT):
                    nc.tensor.matmul(
                        psv[:],
                        x_t[:, kt, tq * P:(tq + 1) * P],
                        wv_r[:, kt, :],
                        start=(kt == 0),
                        stop=(kt == CT - 1),
                    )
                nc.vector.tensor_copy(
                    v_view[:, tt, :, 0:64],
                    psv[:].rearrange("p (h d) -> p h d", h=HL),
                )

        # rstd chain: var = E[x^2]-mu^2, rstd = 1/sqrt(var+eps); chunked
        # so the serial chain pipelines across token groups.
        for g in range(NXQ):
            gs = slice(g * XQ, (g + 1) * XQ)
            tmp = cwork.tile([64, XQ], F32, tag="tmp", bufs=2,
                             name=f"tmp_{rep}_{g}")
            nc.vector.tensor_tensor(tmp[:], sm_mu[:, gs], sm_mu[:, gs], OP.mult)
            nc.vector.scalar_tensor_tensor(
                tmp[:], sm_rst[:, gs], LN_EPS, tmp[:],
                op0=OP.add, op1=OP.subtract,
            )
            nc.scalar.activation(tmp[:], tmp[:], AF.Sqrt)
            nc.vector.reciprocal(sm_rst[:, gs], tmp[:])

    # ============ phases C+D+E: LN apply woven into attention =============
    # C applies LN in place per ft-pair; pair 0 runs inline before the
    # first unit, pairs 1 and 2 ride as fillers inside units 0 and 1 (the
    # units that read them start at unit 2 and 4).  Projection tiles for a
    # finished query chunk ride as fillers in the next chunk's units.
    with ExitStack() as sD:
        psD = sD.enter_context(tc.tile_pool(name=f"psD{rep}", bufs=1, space="PSUM"))

        def c_group(ft, nh):
            """LN apply for one 512-token group of one ft:
            hat = (raw - mu_bcast) * gamma * rstd_bcast [+ beta]."""
            s = ft // 3
            blk = ft % 3
            sl = slice(nh * 512, (nh + 1) * 512)
            bmu = psD.tile([P, 512], F32, tag="scratch", bufs=2,
                           name=f"bmu_{rep}_{ft}_{nh}")
            brs = psD.tile([P, 512], F32, tag="scratch", bufs=2,
                           name=f"brs_{rep}_{ft}_{nh}")
            nc.tensor.matmul(
                bmu[:], bc12[:, ft * P:(ft + 1) * P],
                sm_mu[:, sl], start=True, stop=True,
            )
            nc.tensor.matmul(
                brs[:], bc12[:, ft * P:(ft + 1) * P],
                sm_rst[:, sl], start=True, stop=True,
            )
            tdiff = cwork.tile([P, 512], F32, tag="tdiff", bufs=2,
                               name=f"td_{rep}_{ft}_{nh}")
            nc.vector.tensor_tensor(
                tdiff[:], qk_raw[:, ft, sl], bmu[:], OP.subtract
            )
            nc.vector.scalar_tensor_tensor(
                hat[:, ft, sl],
                tdiff[:],
                gb[:, blk, 2 * s:2 * s + 1],
                brs[:],
                op0=OP.mult,
                op1=OP.mult,
            )
            if not trivial_beta:
                nc.vector.tensor_scalar_add(
                    hat[:, ft, sl], hat[:, ft, sl],
                    gb[:, blk, 2 * s + 1:2 * s + 2],
                )

        def proj_half(pspool, qc, mt, nk):
            """Self-contained projection half-tile: matmuls + copy + DMA."""
            col = slice(qc * QC + nk * 512, qc * QC + (nk + 1) * 512)
            ps_y = pspool.tile([P, 512], F32, tag="scratch", bufs=2,
                               name=f"psy_{rep}_{qc}_{mt}_{nk}")
            for kt in range(KT3):
                nc.tensor.matmul(
                    ps_y[:],
                    wp_r[:, kt, mt * P:(mt + 1) * P],
                    out_t[:, kt, col],
                    start=(kt == 0),
                    stop=(kt == KT3 - 1),
                )
            y_sb = epi.tile([P, 512], F32, tag="y", bufs=2,
                            name=f"y_{rep}_{qc}_{mt}_{nk}")
            nc.vector.tensor_copy(y_sb[:], ps_y[:])
            nc.sync.dma_start(y_d[mt * P:(mt + 1) * P, col], y_sb[:])

        def epilogue(qc, h, ps_o):
            """Stage ps_o to SBUF fast (frees the single PSUM buffer), then
            normalize by the ones-column sum at half granularity."""
            ht = h // 2
            hr = 64 * (h % 2)
            stage = epi.tile([65, QC], F32, tag="stage", bufs=1,
                             name=f"stage_{rep}_{qc}_{h}")
            nc.vector.tensor_copy(stage[:], ps_o[:])
            for hf in range(2):
                hs = slice(hf * 512, (hf + 1) * 512)
                rc = epi.tile([1, 512], F32, tag="rc", bufs=2,
                              name=f"rc_{rep}_{qc}_{h}_{hf}")
                nc.vector.reciprocal(rc[:], stage[64:65, hs])
                rcb = epi.tile([64, 512], F32, tag="rcb", bufs=2,
                               name=f"rcb_{rep}_{qc}_{h}_{hf}")
                nc.gpsimd.partition_broadcast(rcb[:], rc[:])
                nc.vector.tensor_tensor(
                    out_t[hr:hr + 64, ht,
                          qc * QC + hf * 512:qc * QC + (hf + 1) * 512],
                    stage[0:64, hs],
                    rcb[:],
                    OP.mult,
                )

        def mkc(ft, nh):
            return lambda: c_group(ft, nh)

        def mkp(qc, mt, nk):
            return lambda: proj_half(psD, qc, mt, nk)

        # Flat software-pipelined stream over (qc, h, kt): the S/exp stream
        # runs PVLAG slots ahead of the PV stream, continuously across unit
        # boundaries, so ACT always has a full exp queue.  Filler thunks
        # (LN apply groups, projection halves) ride the per-slot PE slack;
        # DVE-heavy C thunks are limited to every other slot.
        s_items = [(qc, h, kt) for qc in range(NQC) for h in range(HL)
                   for kt in range(NKT)]
        n_slots = len(s_items)
        # C groups needed by the very first S matmuls run inline; the rest
        # are queued so each lands well before its first reader.
        c_group(3, 0)
        c_group(0, 0)
        c_group(0, 1)
        fillq = [(mkc(3, 1), True), (mkc(3, 2), True), (mkc(3, 3), True),
                 (mkc(0, 2), True), (mkc(0, 3), True)]
        fillq += [(mkc(ft, nh), True)
                  for ft in (1, 4, 2, 5) for nh in range(N // 512)]
        exps = {}
        ps_o = None
        for i in range(n_slots + PVLAG):
            if i < n_slots:
                qc, h, kt = s_items[i]
                if kt == 2 and qc > 0:
                    # kt==2 (not 0): the previous chunk's last epilogue is
                    # emitted at PV slot lag PVLAG behind; pushing later
                    # guarantees write-before-read order for out_t.
                    fillq += [(mkp(qc - 1, h, nk), False)
                              for nk in range(QC // 512)]
                ht = h // 2
                hr = 64 * (h % 2)
                ps_st = psD.tile([P, QC], F32, tag="ps_s", bufs=2)
                for nk in range(QC // 512):
                    nc.tensor.matmul(
                        ps_st[:, nk * 512:(nk + 1) * 512],
                        hat[hr:hr + 64, 3 + ht, kt * P:(kt + 1) * P],
                        hat[hr:hr + 64, ht,
                            qc * QC + nk * 512:qc * QC + (nk + 1) * 512],
                        start=True,
                        stop=True,
                    )
                exp_t = expp.tile([P, QC], BF16, tag="exp",
                                  name=f"exp_{rep}_{qc}_{h}_{kt}")
                nc.scalar.activation(exp_t[:], ps_st[:], AF.Exp)
                exps[i] = exp_t
            j = i - PVLAG
            if j >= 0:
                qcj, hj, ktj = s_items[j]
                if ktj == 0:
                    ps_o = psD.tile([65, QC], F32, tag="ps_o", bufs=1,
                                    name=f"pso_{rep}_{qcj}_{hj}")
                exp_j = exps.pop(j)
                for nk in range(QC // 512):
                    nc.tensor.matmul(
                        ps_o[:, nk * 512:(nk + 1) * 512],
                        v_view[:, ktj, hj, :],
                        exp_j[:, nk * 512:(nk + 1) * 512],
                        start=(ktj == 0),
                        stop=(ktj == NKT - 1),
                    )
                if ktj == NKT - 1:
                    epilogue(qcj, hj, ps_o)
            if fillq and (not fillq[0][1] or i % 2 == 0):
                fillq.pop(0)[0]()

    def mktail(mt, nk):
        return lambda pspool: _emit_proj_tail(
            nc, rep, pspool, epi, wp_r, out_t, y_d, NQC - 1, mt, nk
        )

    return [mktail(mt, nk) for mt in range(C // P) for nk in range(QC // 512)]


def _emit_proj_tail(nc, rep, pspool, epi, wp_r, out_t, y_d, qc, mt, nk):
    col = slice(qc * QC + nk * 512, qc * QC + (nk + 1) * 512)
    ps_y = pspool.tile([P, 512], F32, tag="scratch", bufs=2,
                       name=f"psyt_{rep}_{qc}_{mt}_{nk}")
    for kt in range(KT3):
        nc.tensor.matmul(
            ps_y[:],
            wp_r[:, kt, mt * P:(mt + 1) * P],
            out_t[:, kt, col],
            start=(kt == 0),
            stop=(kt == KT3 - 1),
        )
    y_sb = epi.tile([P, 512], F32, tag="y", bufs=2,
                    name=f"yt_{rep}_{qc}_{mt}_{nk}")
    nc.vector.tensor_copy(y_sb[:], ps_y[:])
    nc.sync.dma_start(y_d[mt * P:(mt + 1) * P, col], y_sb[:])


def _host_prep(x, w_qkv, q_gamma, q_beta, k_gamma, k_beta, w_proj):
    """Per-core input maps."""
    bd6 = np.zeros((CL, 32), dtype=np.float32)
    for h in range(HL):
        bd6[h * D:(h + 1) * D, h] = 1.0
    # bc12: broadcast selector; for ft, column c -> stat row 32*(ft//3)+(ft%3)*2+c//64
    bc12 = np.zeros((64, FT_QK * P), dtype=np.float32)
    for ft in range(FT_QK):
        s = ft // 3
        blk = ft % 3
        for c in range(P):
            bc12[32 * s + blk * 2 + c // D, ft * P + c] = 1.0
    in_maps = []
    for c in range(8):
        b = c // 2
        half = c % 2
        heads = range(HL * half, HL * half + HL)
        wq = np.concatenate([w_qkv[:, h * D:(h + 1) * D] for h in heads], axis=1)
        wk = np.concatenate(
            [w_qkv[:, C + h * D:C + (h + 1) * D] for h in heads], axis=1
        )
        wv = np.concatenate(
            [w_qkv[:, 2 * C + h * D:2 * C + (h + 1) * D] for h in heads], axis=1
        )
        wqk = np.ascontiguousarray(np.concatenate([wq, wk], axis=1))
        wp = np.ascontiguousarray(w_proj[CL * half:CL * half + CL, :])
        gb = np.stack(
            [
                np.tile(q_gamma, HL) * SCALE,
                np.tile(q_beta, HL) * SCALE,
                np.tile(k_gamma, HL),
                np.tile(k_beta, HL),
            ],
            axis=1,
        ).astype(np.float32)
        in_maps.append(
            {
                "xT": np.ascontiguousarray(x[b].T),
                "wqk": wqk,
                "wv": np.ascontiguousarray(wv),
                "wp": wp,
                "bd6": bd6,
                "bc12": bc12,
                "gb": gb,
            }
        )
    return in_maps


def kernel(x, w_qkv, q_gamma, q_beta, k_gamma, k_beta, w_proj, b_proj):
    x = np.asarray(x, dtype=np.float32)
    w_qkv = np.asarray(w_qkv, dtype=np.float32)
    q_gamma = np.asarray(q_gamma, dtype=np.float32)
    q_beta = np.asarray(q_beta, dtype=np.float32)
    k_gamma = np.asarray(k_gamma, dtype=np.float32)
    k_beta = np.asarray(k_beta, dtype=np.float32)
    w_proj = np.asarray(w_proj, dtype=np.float32)
    b_proj = np.asarray(b_proj, dtype=np.float32)

    trivial_beta = bool(np.all(q_beta == 0.0) and np.all(k_beta == 0.0))
    nc = _build(trivial_beta)
    in_maps = _host_prep(x, w_qkv, q_gamma, q_beta, k_gamma, k_beta, w_proj)
    res = run_bass_kernel_spmd(nc, in_maps, core_ids=list(range(8)))

    y = np.empty((B, N, C), dtype=np.float32)
    for b in range(B):
        yt = res.results[2 * b]["y"] + res.results[2 * b + 1]["y"]
        y[b] = yt.T + b_proj[None, :]
    return y


if __name__ == "__main__":
    rng = np.random.default_rng(0)
    out = kernel(
        rng.standard_normal((B, N, C), dtype=np.float32),
        (rng.standard_normal((C, 3 * C)) * C ** -0.5).astype(np.float32),
        np.ones(D, np.float32),
        np.zeros(D, np.float32),
        np.ones(D, np.float32),
        np.zeros(D, np.float32),
        (rng.standard_normal((C, C)) * C ** -0.5).astype(np.float32),
        np.zeros(C, np.float32),
    )
    print("ok", out.shape, float(np.abs(out).mean()))


# revision 39
# speedup vs baseline: 1.7440x; 1.7440x over previous
"""Trainium2 Bass kernel for nn_Attention_45835890982922.

Dense multi-head attention block:
    qkv = x @ w_qkv ; q,k layernormed per head (eps=1e-5), q scaled by D^-0.5
    out = softmax(q k^T) v ; y = concat_heads(out) @ w_proj + b_proj

Sharding over 8 NeuronCores: hybrid batch x tensor-parallel.
Core c handles batch b = c//2 and heads [6*(c%2), 6*(c%2)+6).
Each core computes a partial y^T (its 6 heads through the matching
w_proj rows); the host sums the two partials per batch and adds b_proj.

On-chip layout is feature-major (transposed): the host ships x^T per
core, so every matmul contraction lives on the partition axis with no
on-chip transposes.  Softmax runs without max-subtraction (|S| <= ~8
after LN), with the normalization sum obtained from an extra all-ones
column appended to v; the division is folded into the epilogue of the
attention-output matmul.

Pipeline structure (per iteration):
  A: x^T streamed in 512-token quarters; each quarter does qkv matmuls,
     LN stat matmuls, and v matmuls before the next quarter's DMA lands.
  C: rstd chain + LN apply in place.
  D+E fused: per (query-chunk, head) unit, PV matmuls run two key-tiles
     behind S/exp inside the same unit (so ACT always has exp work while
     the PE alternates S and PV); the output projection for a finished
     query chunk DMAs straight from PSUM to DRAM.

All SBUF pools are persistent across repeat iterations so steady-state
runs prefetch the next iteration's x while attention drains.

dtypes: float32r (TensorE reduced fp32, ~1.5e-4) for qkv/S/stats/proj
matmuls, bf16 for exp(S) probabilities and v, fp32 accumulation in PSUM.
"""

from contextlib import ExitStack

import numpy as np

import concourse.bacc as bacc
import concourse.tile as tile
import concourse.mybir as mybir
from concourse.bass_utils import run_bass_kernel_spmd

F32 = mybir.dt.float32
F32R = mybir.dt.float32r
BF16 = mybir.dt.bfloat16
OP = mybir.AluOpType
AF = mybir.ActivationFunctionType

B, N, C, H, D = 4, 2048, 768, 12, 64
HL = H // 2              # 6 heads per core
CL = HL * D              # 384 local feature rows
P = 128
NKT = N // P             # 16 key tiles
QC = 1024                # query chunk for attention
NQC = N // QC
CT = C // P              # 6 contraction tiles over C
FT_QK = 2 * CL // P      # 6 feature tiles for q|k
KT3 = CL // P            # 3 contraction tiles over CL
LN_EPS = 1e-5
SCALE = float(D) ** -0.5
XQ = 512                 # x^T streaming quarter (tokens)
NXQ = N // XQ
PVLAG = 2                # PV runs this many key-tiles behind S/exp


def _build(trivial_beta: bool, repeat: int = 1):
    nc = bacc.Bacc("TRN2", target_bir_lowering=False, debug=False, num_devices=8)

    xT_d = nc.dram_tensor("xT", [C, N], F32R, kind="ExternalInput").ap()
    wqk_d = nc.dram_tensor("wqk", [C, 2 * CL], F32R, kind="ExternalInput").ap()
    wv_d = nc.dram_tensor("wv", [C, CL], F32R, kind="ExternalInput").ap()
    wp_d = nc.dram_tensor("wp", [CL, C], F32R, kind="ExternalInput").ap()
    bd6_d = nc.dram_tensor("bd6", [CL, 32], F32R, kind="ExternalInput").ap()
    bc12_d = nc.dram_tensor("bc12", [64, FT_QK * P], F32R, kind="ExternalInput").ap()
    gb_d = nc.dram_tensor("gb", [CL, 4], F32, kind="ExternalInput").ap()
    y_d = nc.dram_tensor("y", [C, N], F32, kind="ExternalOutput").ap()

    with tile.TileContext(nc) as tc, ExitStack() as top:
        top.enter_context(
            nc.allow_low_precision(reason="f32r/bf16 staging is intentional")
        )
        const = top.enter_context(tc.tile_pool(name="const", bufs=1))
        bd6 = const.tile([P, KT3, 32], F32R)
        nc.sync.dma_start(bd6[:], bd6_d.rearrange("(t p) h -> p t h", p=P))
        bc12 = const.tile([64, FT_QK * P], F32R)
        nc.sync.dma_start(bc12[:], bc12_d)
        gb = const.tile([P, KT3, 4], F32)
        nc.sync.dma_start(gb[:], gb_d.rearrange("(t p) c -> p t c", p=P))

        # persistent state: weights (loaded once), qk/v/out accumulators,
        # LN stats, x^T streaming ring, exp ring, epilogue smalls.
        pers = top.enter_context(tc.tile_pool(name="pers", bufs=1))
        wqk_r = pers.tile([P, CT, 2 * CL], F32R)
        nc.sync.dma_start(wqk_r[:], wqk_d.rearrange("(t p) f -> p t f", p=P))
        wv_r = pers.tile([P, CT, CL], F32R)
        nc.sync.dma_start(wv_r[:], wv_d.rearrange("(t p) f -> p t f", p=P))
        wp_r = pers.tile([P, KT3, C], F32R)
        nc.sync.dma_start(wp_r[:], wp_d.rearrange("(t p) f -> p t f", p=P))

        # v token-major bf16 with per-head all-ones column: [p, ttile, h*65+e]
        v_sb = pers.tile([P, NKT, HL * 65], BF16)
        v_view = v_sb[:].rearrange("p t (h e) -> p t h e", h=HL)
        nc.gpsimd.memset(v_view[:, :, :, 64:65], 1.0)

        qk_fts = [pers.tile([P, N], F32R, name=f"qk_ft{ft}") for ft in range(FT_QK)]
        out_fts = [pers.tile([P, N], F32R, name=f"out_ft{t}") for t in range(KT3)]
        # combined stats: rows 0-5 = q heads, rows 32-37 = k heads; the
        # unused middle rows are zeroed once so full-partition reads (chain
        # ops, bcast matmuls against zero bc12 rows) never see garbage.
        sm_mu = pers.tile([64, N], F32R, name="sm_mu")
        sm_rst = pers.tile([64, N], F32R, name="sm_rst")


        xtp = top.enter_context(tc.tile_pool(name="xtp", bufs=2))
        sqp = top.enter_context(tc.tile_pool(name="sqp", bufs=2))
        expp = top.enter_context(tc.tile_pool(name="expp", bufs=4 + PVLAG))
        epi = top.enter_context(tc.tile_pool(name="epi", bufs=1))
        cwork = top.enter_context(tc.tile_pool(name="cwork", bufs=2))

        state = dict(
            xT_d=xT_d, y_d=y_d, bd6=bd6, bc12=bc12, gb=gb,
            wqk_r=wqk_r, wv_r=wv_r, wp_r=wp_r,
            v_view=v_view, qk_fts=qk_fts, out_fts=out_fts,
            sm_mu=sm_mu, sm_rst=sm_rst,
            xtp=xtp, sqp=sqp, expp=expp, epi=epi, cwork=cwork,
        )
        pending = None
        for rep in range(repeat):
            pending = _emit_iteration(nc, tc, rep, trivial_beta, state, pending)
        with tc.tile_pool(name="psT", bufs=1, space="PSUM") as psT:
            for t in pending:
                t(psT)

    nc.compile()
    return nc


class _FtView:
    """view[p_slice, ft, col_slice] -> per-ft tile AP."""
    def __init__(self, tiles):
        self.tiles = tiles

    def __getitem__(self, idx):
        p, ft, col = idx
        return self.tiles[ft][p, col]


def _emit_iteration(nc, tc, rep, trivial_beta, st, pending=None):
    xT_d, y_d = st["xT_d"], st["y_d"]
    bd6, bc12, gb = st["bd6"], st["bc12"], st["gb"]
    wqk_r, wv_r, wp_r = st["wqk_r"], st["wv_r"], st["wp_r"]
    v_view = st["v_view"]
    sm_mu, sm_rst = st["sm_mu"], st["sm_rst"]
    xtp, sqp, expp, epi = st["xtp"], st["sqp"], st["expp"], st["epi"]
    cwork = st["cwork"]
    qk_raw = _FtView(st["qk_fts"])
    hat = qk_raw
    out_t = _FtView(st["out_fts"])

    # ================ phase A: x^T quarters -> qk, LN stats, v ================
    with ExitStack() as sA:
        psA = sA.enter_context(tc.tile_pool(name=f"psA{rep}", bufs=2, space="PSUM"))
        for g in range(NXQ):
            gs = slice(g * XQ, (g + 1) * XQ)
            x_t = xtp.tile([P, CT, XQ], F32R, tag="xt", name=f"xt_{rep}_{g}")
            nc.sync.dma_start(
                x_t[:], xT_d.rearrange("(t p) n -> p t n", p=P)[:, :, gs]
            )
            for ft in range(FT_QK):
                ps = psA.tile([P, XQ], F32, tag="ps_qkv")
                for kt in range(CT):
                    nc.tensor.matmul(
                        ps[:],
                        wqk_r[:, kt, ft * P:(ft + 1) * P],
                        x_t[:, kt, :],
                        start=(kt == 0),
                        stop=(kt == CT - 1),
                    )
                nc.vector.tensor_copy(qk_raw[:, ft, gs], ps[:])
                if pending:
                    pending.pop(0)(psA)
            # LN stat sums over D via block-diagonal ones matmuls; q and k
            # accumulate in separate base-0 PSUM tiles, merged at the copy.
            for s in range(2):
                psm = psA.tile([32, XQ], F32, tag="ps_stat")
                for kt in range(KT3):
                    nc.tensor.matmul(
                        psm[:],
                        bd6[:, kt, :],
                        qk_raw[:, 3 * s + kt, gs],
                        start=(kt == 0),
                        stop=(kt == KT3 - 1),
                    )
                nc.vector.tensor_scalar_mul(
                    sm_mu[32 * s:32 * s + 32, gs], psm[:], 1.0 / D
                )
            for s in range(2):
                psm2 = psA.tile([32, XQ], F32, tag="ps_stat")
                for kt in range(KT3):
                    sq = sqp.tile([P, XQ], F32R, tag="sq", name=f"sq_{rep}_{g}_{s}_{kt}")
                    nc.scalar.square(sq[:], qk_raw[:, 3 * s + kt, gs])
                    nc.tensor.matmul(
                        psm2[:],
                        bd6[:, kt, :],
                        sq[:],
                        start=(kt == 0),
                        stop=(kt == KT3 - 1),
                    )
                nc.vector.tensor_scalar_mul(
                    sm_rst[32 * s:32 * s + 32, gs], psm2[:], 1.0 / D
                )
            # v matmuls for this quarter's token tiles
            for tq in range(XQ // P):
                tt = g * (XQ // P) + tq
                psv = psA.tile([P, CL], F32, tag="ps_v")
                for kt in range(CT):
                    nc.tensor.matmul(
                        psv[:],
                        x_t[:, kt, tq * P:(tq + 1) * P],
                        wv_r[:, kt, :],
                        start=(kt == 0),
                        stop=(kt == CT - 1),
                    )
                nc.vector.tensor_copy(
                    v_view[:, tt, :, 0:64],
                    psv[:].rearrange("p (h d) -> p h d", h=HL),
                )

        # rstd chain: var = E[x^2]-mu^2, rstd = 1/sqrt(var+eps); chunked
        # so the serial chain pipelines across token groups.
        for g in range(NXQ):
            gs = slice(g * XQ, (g + 1) * XQ)
            tmp = cwork.tile([64, XQ], F32, tag="tmp", bufs=2,
                             name=f"tmp_{rep}_{g}")
            nc.vector.tensor_tensor(tmp[:], sm_mu[:, gs], sm_mu[:, gs], OP.mult)
            nc.vector.scalar_tensor_tensor(
                tmp[:], sm_rst[:, gs], LN_EPS, tmp[:],
                op0=OP.add, op1=OP.subtract,
            )
            nc.scalar.activation(tmp[:], tmp[:], AF.Sqrt)
            nc.vector.reciprocal(sm_rst[:, gs], tmp[:])

    # ============ phases C+D+E: LN apply woven into attention =============
    # C applies LN in place per ft-pair; pair 0 runs inline before the
    # first unit, pairs 1 and 2 ride as fillers inside units 0 and 1 (the
    # units that read them start at unit 2 and 4).  Projection tiles for a
    # finished query chunk ride as fillers in the next chunk's units.
    with ExitStack() as sD:
        psD = sD.enter_context(tc.tile_pool(name=f"psD{rep}", bufs=1, space="PSUM"))

        def c_group(ft, nh):
            """LN apply for one 512-token group of one ft:
            hat = (raw - mu_bcast) * gamma * rstd_bcast [+ beta]."""
            s = ft // 3
            blk = ft % 3
            sl = slice(nh * 512, (nh + 1) * 512)
            bmu = psD.tile([P, 512], F32, tag="scratch", bufs=2,
                           name=f"bmu_{rep}_{ft}_{nh}")
            brs = psD.tile([P, 512], F32, tag="scratch", bufs=2,
                           name=f"brs_{rep}_{ft}_{nh}")
            nc.tensor.matmul(
                bmu[:], bc12[:, ft * P:(ft + 1) * P],
                sm_mu[:, sl], start=True, stop=True,
            )
            nc.tensor.matmul(
                brs[:], bc12[:, ft * P:(ft + 1) * P],
                sm_rst[:, sl], start=True, stop=True,
            )
            tdiff = cwork.tile([P, 512], F32, tag="tdiff", bufs=2,
                               name=f"td_{rep}_{ft}_{nh}")
            nc.vector.tensor_tensor(
                tdiff[:], qk_raw[:, ft, sl], bmu[:], OP.subtract
            )
            nc.vector.scalar_tensor_tensor(
                hat[:, ft, sl],
                tdiff[:],
                gb[:, blk, 2 * s:2 * s + 1],
                brs[:],
                op0=OP.mult,
                op1=OP.mult,
            )
            if not trivial_beta:
                nc.vector.tensor_scalar_add(
                    hat[:, ft, sl], hat[:, ft, sl],
                    gb[:, blk, 2 * s + 1:2 * s + 2],
                )

        def proj_half(pspool, qc, mt, nk):
            """Self-contained projection half-tile: matmuls + copy + DMA."""
            col = slice(qc * QC + nk * 512, qc * QC + (nk + 1) * 512)
            ps_y = pspool.tile([P, 512], F32, tag="scratch", bufs=2,
                               name=f"psy_{rep}_{qc}_{mt}_{nk}")
            for kt in range(KT3):
                nc.tensor.matmul(
                    ps_y[:],
                    wp_r[:, kt, mt * P:(mt + 1) * P],
                    out_t[:, kt, col],
                    start=(kt == 0),
                    stop=(kt == KT3 - 1),
                )
            y_sb = epi.tile([P, 512], F32, tag="y", bufs=2,
                            name=f"y_{rep}_{qc}_{mt}_{nk}")
            nc.vector.tensor_copy(y_sb[:], ps_y[:])
            nc.sync.dma_start(y_d[mt * P:(mt + 1) * P, col], y_sb[:])

        def epilogue(qc, h, ps_o):
            """Stage ps_o to SBUF fast (frees the single PSUM buffer), then
            normalize by the ones-column sum at half granularity."""
            ht = h // 2
            hr = 64 * (h % 2)
            stage = epi.tile([65, QC], F32, tag="stage", bufs=1,
                             name=f"stage_{rep}_{qc}_{h}")
            nc.vector.tensor_copy(stage[:], ps_o[:])
            for hf in range(2):
                hs = slice(hf * 512, (hf + 1) * 512)
                rc = epi.tile([1, 512], F32, tag="rc", bufs=2,
                              name=f"rc_{rep}_{qc}_{h}_{hf}")
                nc.vector.reciprocal(rc[:], stage[64:65, hs])
                rcb = epi.tile([64, 512], F32, tag="rcb", bufs=2,
                               name=f"rcb_{rep}_{qc}_{h}_{hf}")
                nc.gpsimd.partition_broadcast(rcb[:], rc[:])
                nc.vector.tensor_tensor(
                    out_t[hr:hr + 64, ht,
                          qc * QC + hf * 512:qc * QC + (hf + 1) * 512],
                    stage[0:64, hs],
                    rcb[:],
                    OP.mult,
                )

        def mkc(ft, nh):
            return lambda: c_group(ft, nh)

        def mkp(qc, mt, nk):
            return lambda: proj_half(psD, qc, mt, nk)

        # Flat software-pipelined stream over (qc, h, kt): the S/exp stream
        # runs PVLAG slots ahead of the PV stream, continuously across unit
        # boundaries, so ACT always has a full exp queue.  Filler thunks
        # (LN apply groups, projection halves) ride the per-slot PE slack,
        # one per slot, draining the LN-apply queue as early as possible.
        s_items = [(qc, h, kt) for qc in range(NQC) for h in range(HL)
                   for kt in range(NKT)]
        n_slots = len(s_items)
        # C groups needed by the very first S matmuls run inline; the rest
        # are queued so each lands well before its first reader.
        c_group(3, 0)
        c_group(0, 0)
        c_group(0, 1)
        fillq = [(mkc(3, 1), True), (mkc(3, 2), True), (mkc(3, 3), True),
                 (mkc(0, 2), True), (mkc(0, 3), True)]
        fillq += [(mkc(ft, nh), True)
                  for ft in (1, 4, 2, 5) for nh in range(N // 512)]
        exps = {}
        ps_o = None
        for i in range(n_slots + PVLAG):
            if i < n_slots:
                qc, h, kt = s_items[i]
                if kt == 2 and qc > 0:
                    # kt==2 (not 0): the previous chunk's last epilogue is
                    # emitted at PV slot lag PVLAG behind; pushing later
                    # guarantees write-before-read order for out_t.
                    fillq += [(mkp(qc - 1, h, nk), False)
                              for nk in range(QC // 512)]
                ht = h // 2
                hr = 64 * (h % 2)
                ps_st = psD.tile([P, QC], F32, tag="ps_s", bufs=2)
                for nk in range(QC // 512):
                    nc.tensor.matmul(
                        ps_st[:, nk * 512:(nk + 1) * 512],
                        hat[hr:hr + 64, 3 + ht, kt * P:(kt + 1) * P],
                        hat[hr:hr + 64, ht,
                            qc * QC + nk * 512:qc * QC + (nk + 1) * 512],
                        start=True,
                        stop=True,
                    )
                exp_t = expp.tile([P, QC], BF16, tag="exp",
                                  name=f"exp_{rep}_{qc}_{h}_{kt}")
                nc.scalar.activation(exp_t[:], ps_st[:], AF.Exp)
                exps[i] = exp_t
            j = i - PVLAG
            if j >= 0:
                qcj, hj, ktj = s_items[j]
                if ktj == 0:
                    ps_o = psD.tile([65, QC], F32, tag="ps_o", bufs=1,
                                    name=f"pso_{rep}_{qcj}_{hj}")
                exp_j = exps.pop(j)
                for nk in range(QC // 512):
                    nc.tensor.matmul(
                        ps_o[:, nk * 512:(nk + 1) * 512],
                        v_view[:, ktj, hj, :],
                        exp_j[:, nk * 512:(nk + 1) * 512],
                        start=(ktj == 0),
                        stop=(ktj == NKT - 1),
                    )
                if ktj == NKT - 1:
                    epilogue(qcj, hj, ps_o)
            if fillq:
                fillq.pop(0)[0]()

    def mktail(mt, nk):
        return lambda pspool: _emit_proj_tail(
            nc, rep, pspool, epi, wp_r, out_t, y_d, NQC - 1, mt, nk
        )

    return [mktail(mt, nk) for mt in range(C // P) for nk in range(QC // 512)]


def _emit_proj_tail(nc, rep, pspool, epi, wp_r, out_t, y_d, qc, mt, nk):
    col = slice(qc * QC + nk * 512, qc * QC + (nk + 1) * 512)
    ps_y = pspool.tile([P, 512], F32, tag="scratch", bufs=2,
                       name=f"psyt_{rep}_{qc}_{mt}_{nk}")
    for kt in range(KT3):
        nc.tensor.matmul(
            ps_y[:],
            wp_r[:, kt, mt * P:(mt + 1) * P],
            out_t[:, kt, col],
            start=(kt == 0),
            stop=(kt == KT3 - 1),
        )
    y_sb = epi.tile([P, 512], F32, tag="y", bufs=2,
                    name=f"yt_{rep}_{qc}_{mt}_{nk}")
    nc.vector.tensor_copy(y_sb[:], ps_y[:])
    nc.sync.dma_start(y_d[mt * P:(mt + 1) * P, col], y_sb[:])


def _host_prep(x, w_qkv, q_gamma, q_beta, k_gamma, k_beta, w_proj):
    """Per-core input maps."""
    bd6 = np.zeros((CL, 32), dtype=np.float32)
    for h in range(HL):
        bd6[h * D:(h + 1) * D, h] = 1.0
    # bc12: broadcast selector; for ft, column c -> stat row 32*(ft//3)+(ft%3)*2+c//64
    bc12 = np.zeros((64, FT_QK * P), dtype=np.float32)
    for ft in range(FT_QK):
        s = ft // 3
        blk = ft % 3
        for c in range(P):
            bc12[32 * s + blk * 2 + c // D, ft * P + c] = 1.0
    in_maps = []
    for c in range(8):
        b = c // 2
        half = c % 2
        heads = range(HL * half, HL * half + HL)
        wq = np.concatenate([w_qkv[:, h * D:(h + 1) * D] for h in heads], axis=1)
        wk = np.concatenate(
            [w_qkv[:, C + h * D:C + (h + 1) * D] for h in heads], axis=1
        )
        wv = np.concatenate(
            [w_qkv[:, 2 * C + h * D:2 * C + (h + 1) * D] for h in heads], axis=1
        )
        wqk = np.ascontiguousarray(np.concatenate([wq, wk], axis=1))
        wp = np.ascontiguousarray(w_proj[CL * half:CL * half + CL, :])
        gb = np.stack(
            [
                np.tile(q_gamma, HL) * SCALE,
                np.tile(q_beta, HL) * SCALE,
                np.tile(k_gamma, HL),
                np.tile(k_beta, HL),
            ],
            axis=1,
        ).astype(np.float32)
        in_maps.append(
            {
                "xT": np.ascontiguousarray(x[b].T),
                "wqk": wqk,
                "wv": np.ascontiguousarray(wv),
                "wp": wp,
                "bd6": bd6,
                "bc12": bc12,
                "gb": gb,
            }
        )
    return in_maps


def kernel(x, w_qkv, q_gamma, q_beta, k_gamma, k_beta, w_proj, b_proj):
    x = np.asarray(x, dtype=np.float32)
    w_qkv = np.asarray(w_qkv, dtype=np.float32)
    q_gamma = np.asarray(q_gamma, dtype=np.float32)
    q_beta = np.asarray(q_beta, dtype=np.float32)
    k_gamma = np.asarray(k_gamma, dtype=np.float32)
    k_beta = np.asarray(k_beta, dtype=np.float32)
    w_proj = np.asarray(w_proj, dtype=np.float32)
    b_proj = np.asarray(b_proj, dtype=np.float32)

    trivial_beta = bool(np.all(q_beta == 0.0) and np.all(k_beta == 0.0))
    nc = _build(trivial_beta)
    in_maps = _host_prep(x, w_qkv, q_gamma, q_beta, k_gamma, k_beta, w_proj)
    res = run_bass_kernel_spmd(nc, in_maps, core_ids=list(range(8)))

    y = np.empty((B, N, C), dtype=np.float32)
    for b in range(B):
        yt = res.results[2 * b]["y"] + res.results[2 * b + 1]["y"]
        y[b] = yt.T + b_proj[None, :]
    return y


if __name__ == "__main__":
    rng = np.random.default_rng(0)
    out = kernel(
        rng.standard_normal((B, N, C), dtype=np.float32),
        (rng.standard_normal((C, 3 * C)) * C ** -0.5).astype(np.float32),
        np.ones(D, np.float32),
        np.zeros(D, np.float32),
        np.ones(D, np.float32),
        np.zeros(D, np.float32),
        (rng.standard_normal((C, C)) * C ** -0.5).astype(np.float32),
        np.zeros(C, np.float32),
    )
    print("ok", out.shape, float(np.abs(out).mean()))
